# revision 14
# baseline (speedup 1.0000x reference)
"""Depthwise-separable conv (3x3 depthwise rank-1 + 1x1 pointwise) on 8
Trainium2 NeuronCores.

Sharding: data-parallel over batch — 2 images per core.

Per-core algorithm, per 32-row slab (C=128 channels on partitions):
  1. DMA the slab of x (with 1-row halo) into SBUF.
  2. Column conv (3-tap along H, per-channel scalars): ACT does the center
     tap (activation Copy with per-partition scale), DVE accumulates the
     two shifted taps in-place via scalar_tensor_tensor. Output y1 is
     rounded to float32r (required for the fast fp32r matmul path).
  3. Row conv + pointwise are folded into the PE: out[o,h,w] =
     sum_j (pw[o,c]*row[c,j]) y1[c,h,w+j-1] — 3 accumulated float32r
     matmuls per 512-element PSUM chunk, with w-shifts expressed as
     offset access patterns (edge columns get zero-pad semantics for free
     by narrowing the APs).
  4. PSUM chunks are evacuated to SBUF by ACT/DVE (alternating), then
     DMA'd to the output.
"""
import sys

sys.path.insert(0, "/opt/trn_rl_repo")

from contextlib import ExitStack

import numpy as np

import concourse.bass as bass
import concourse.tile as tile
from concourse import bacc, mybir
from concourse.bass_utils import run_bass_kernel_spmd

F32 = mybir.dt.float32
F32R = mybir.dt.float32r

B, C, H, W = 16, 128, 128, 128
OUT = 256
N_CORES = 8
B_LOC = B // N_CORES          # images per core
SLAB = 32                     # rows per slab (x DMA granularity)
N_SLABS = H // SLAB
SUB = 8                       # rows per col-pass sub-slab
N_SUB = SLAB // SUB
CHUNK = 512                   # psum chunk = 4 rows of W=128
N_CHUNK = SLAB * W // CHUNK   # 8 chunks per slab

LAST_EXEC_NS = None

_CACHED_NC = None


def _build():
    nc = bacc.Bacc(trn_type="TRN2", target_bir_lowering=False, debug=False)
    xin = nc.dram_tensor("xin", [B_LOC, C, H, W], F32, kind="ExternalInput").ap()
    wfold = nc.dram_tensor("wfold", [3, C, OUT], F32, kind="ExternalInput").ap()
    colk = nc.dram_tensor("colk", [C, 3], F32, kind="ExternalInput").ap()
    out = nc.dram_tensor("out", [B_LOC, OUT, H, W], F32, kind="ExternalOutput").ap()

    with tile.TileContext(nc) as tc, ExitStack() as ctx:
        wpool = ctx.enter_context(tc.tile_pool(name="weights", bufs=1))
        xpool = ctx.enter_context(tc.tile_pool(name="x", bufs=5))
        ypool = ctx.enter_context(tc.tile_pool(name="y1", bufs=6))
        opool = ctx.enter_context(tc.tile_pool(name="out", bufs=6))
        pspool = ctx.enter_context(tc.tile_pool(name="ps", bufs=8, space="PSUM"))

        # --- weights: DMA fp32, round to f32r on DVE (fp32r matmul operands
        # must be produced by a rounding compute op, not a DMA)
        w_f32 = wpool.tile([C, 3 * OUT], F32, tag="w32")
        for j in range(3):
            nc.sync.dma_start(w_f32[:, j * OUT:(j + 1) * OUT], wfold[j])
        w_r = wpool.tile([C, 3 * OUT], F32R, tag="wr")
        nc.vector.tensor_copy(w_r[:], w_f32[:])
        ck = wpool.tile([C, 3], F32, tag="ck")
        nc.sync.dma_start(ck[:], colk[:])

        def wj(j, oc):  # lhsT [C=128, O=128] for tap j, out-channel half oc
            return w_r[:, j * OUT + oc * 128: j * OUT + oc * 128 + 128]

        for b in range(B_LOC):
            for s in range(N_SLABS):
                h0 = s * SLAB
                # --- x slab with halo: rows h0-1 .. h0+SLAB (SLAB+2 rows)
                x_t = xpool.tile([C, (SLAB + 2) * W], F32, tag="xs")
                if s == 0:
                    nc.gpsimd.memset(x_t[:, 0:W], 0.0)
                    nc.sync.dma_start(x_t[:, W:], xin[b, :, 0:SLAB + 1, :])
                elif s == N_SLABS - 1:
                    nc.gpsimd.memset(x_t[:, (SLAB + 1) * W:], 0.0)
                    nc.sync.dma_start(x_t[:, 0:(SLAB + 1) * W],
                                      xin[b, :, h0 - 1:H, :])
                else:
                    nc.sync.dma_start(x_t[:], xin[b, :, h0 - 1:h0 + SLAB + 1, :])

                # --- process in SUB-row sub-slabs so matmuls can start as
                # soon as a sub-slab's column pass is done (keeps PE dense
                # and warm instead of stalling on the whole-slab y1 chain).
                x3 = x_t[:].rearrange("c (h w) -> c h w", w=W)
                WP = W + 1
                RPC = CHUNK // W                  # rows per psum chunk (4)
                # out staging: half-slab tiles, DMA'd as soon as filled
                ot = [[opool.tile([C, SLAB // 2 * W], F32, tag="ot",
                                  name=f"ot_{b}_{s}_{oc}_{hh}")
                       for hh in range(2)] for oc in range(2)]  # [oc][half]

                for ss in range(N_SUB):
                    base = ss * SUB
                    # y1 sub-tile: SUB data rows, row stride W+1; y1[h][w]
                    # lives at offset 1 + h*WP + w, and the inter-row pad
                    # column (offset h*WP) is zeroed => the row conv gets
                    # zero-pad edge semantics with full-width, fp32r-legal
                    # access patterns: tap j of rows r0..r0+3 is
                    # ypj[j][:, r0:r0+4, 0:W].
                    y1 = ypool.tile([C, (SUB + 1) * WP + 2], F32R, tag="y1")
                    yp = y1[:, 0:(SUB + 1) * WP].rearrange(
                        "c (h w) -> c h w", w=WP)
                    ypj = [y1[:, j:j + (SUB + 1) * WP]
                           .rearrange("c (h w) -> c h w", w=WP)
                           for j in range(3)]
                    nc.vector.memset(yp[:, :, 0:1].bitcast(F32), 0.0)
                    yd = yp[:, 0:SUB, 1:WP]       # data view [C, SUB, W]
                    nc.scalar.activation(yd, x3[:, base + 1:base + SUB + 1, :],
                                         mybir.ActivationFunctionType.Copy,
                                         scale=ck[:, 1:2])
                    nc.vector.scalar_tensor_tensor(
                        yd, x3[:, base:base + SUB, :], ck[:, 0:1], yd,
                        op0=mybir.AluOpType.mult, op1=mybir.AluOpType.add)
                    nc.vector.scalar_tensor_tensor(
                        yd, x3[:, base + 2:base + SUB + 2, :], ck[:, 2:3], yd,
                        op0=mybir.AluOpType.mult, op1=mybir.AluOpType.add)

                    # row conv + pointwise folded into PE (f32r matmuls)
                    for oc in range(2):
                        for qq in range(SUB // RPC):
                            ps = pspool.tile([128, CHUNK], F32, tag="ps")
                            r0 = qq * RPC
                            for jx, j in enumerate((0, 1, 2)):
                                nc.tensor.matmul(
                                    ps[:], wj(j, oc),
                                    ypj[j][:, r0:r0 + RPC, 0:W],
                                    start=(jx == 0), stop=(jx == 2))
                            qg = ss * (SUB // RPC) + qq   # global chunk 0..7
                            half = qg // (N_CHUNK // 2)
                            qh = qg % (N_CHUNK // 2)
                            dst = ot[oc][half][:, qh * CHUNK:(qh + 1) * CHUNK]
                            if (oc * (SUB // RPC) + qq) % 4 == 3:
                                nc.vector.tensor_copy(dst, ps[:])
                            else:
                                nc.scalar.copy(dst, ps[:])
                    if ss % (N_SUB // 2) == N_SUB // 2 - 1:
                        half = ss // (N_SUB // 2)
                        hr = h0 + half * (SLAB // 2)
                        for oc in range(2):
                            # issue from GpSimd (SWDGE) so output DMAs never
                            # block the SP input-prefetch issue stream
                            nc.gpsimd.dma_start(
                                out[b, oc * 128:(oc + 1) * 128,
                                    hr:hr + SLAB // 2, :], ot[oc][half][:])
    nc.compile()
    return nc


def kernel(x, col_kernel, row_kernel, pw_weight, trace=False):
    global LAST_EXEC_NS, _CACHED_NC
    x = np.ascontiguousarray(np.asarray(x, dtype=np.float32))
    colk3 = np.asarray(col_kernel, dtype=np.float32).reshape(C, 3)
    rowk3 = np.asarray(row_kernel, dtype=np.float32).reshape(C, 3)
    pw = np.asarray(pw_weight, dtype=np.float32)

    # fold row-conv taps into the pointwise weight: Wj[c, o] = pw[o,c]*row[c,j]
    wfold = np.ascontiguousarray(
        pw.T[None, :, :] * rowk3.T[:, :, None]).astype(np.float32)  # [3, C, OUT]

    if _CACHED_NC is None:
        _CACHED_NC = _build()
    nc = _CACHED_NC

    in_maps = [
        {"xin": np.ascontiguousarray(x[i * B_LOC:(i + 1) * B_LOC]),
         "wfold": wfold, "colk": np.ascontiguousarray(colk3)}
        for i in range(N_CORES)
    ]
    res = run_bass_kernel_spmd(nc, in_maps, list(range(N_CORES)), trace=trace)
    LAST_EXEC_NS = res.exec_time_ns
    return np.concatenate([res.results[i]["out"] for i in range(N_CORES)],
                          axis=0)


# revision 32
# speedup vs baseline: 4.3748x; 4.3748x over previous
"""Depthwise-separable conv (3x3 depthwise rank-1 + 1x1 pointwise) on 8
Trainium2 NeuronCores.

Sharding: data-parallel over batch — 2 images per core. The kernel is
memory-bound (reads 16 MiB of x, writes 32 MiB of out per core); measured
steady-state ~93 us/core, at the machine's DMA roofline for that traffic.

Per-core algorithm, per 32-row slab (C=128 channels on partitions),
processed in 8-row sub-slabs for fine-grained pipelining:
  1. DMA the x slab (with 1-row halo) into SBUF (SP/HWDGE issue stream
     carries ONLY input prefetch so it never blocks behind compute).
  2. Column conv (3 taps along H, per-channel scalars) in TWO DVE
     scalar_tensor_tensor ops: y1' = (x_up*a0 + x_center) + x_down*a2,
     where a_i = col_i/col_1 and col_1 is folded into the matmul weights
     on the host. y1' is written as float32r (required producer rounding
     for the fast fp32r matmul path) with a 129-element row stride whose
     zeroed inter-row pad column provides zero-pad edge semantics.
  3. Row conv + pointwise folded into the PE: out[o,h,w] =
     sum_j (pw[o,c]*row[c,j]*col1[c]) y1'[c,h,w+j-1] — 3 accumulated
     float32r matmuls (full speed: 1 cycle/row at N=512) per PSUM bank,
     w-shifts expressed as +j access-pattern offsets into the padded y1'.
  4. PSUM banks are evacuated to SBUF staging mostly by ACT (DVE takes
     1/16), then DMA'd out — issued from ACT (HWDGE) for tiles ACT
     evacuated itself, GpSimd (SWDGE) for the DVE-evacuated tile.
"""
import sys

sys.path.insert(0, "/opt/trn_rl_repo")

from contextlib import ExitStack

import numpy as np

import concourse.bass as bass
import concourse.tile as tile
from concourse import bacc, mybir
from concourse.bass_utils import run_bass_kernel_spmd

F32 = mybir.dt.float32
F32R = mybir.dt.float32r

B, C, H, W = 16, 128, 128, 128
OUT = 256
N_CORES = 8
B_LOC = B // N_CORES          # images per core
SLAB = 32                     # rows per slab (x DMA granularity)
N_SLABS = H // SLAB
SUB = 8                       # rows per col-pass sub-slab
N_SUB = SLAB // SUB
CHUNK = 512                   # psum chunk = 4 rows of W=128
N_CHUNK = SLAB * W // CHUNK   # 8 chunks per slab

LAST_EXEC_NS = None

_CACHED_NC = None


def _build(repeat=1, factored=True):
    """factored=True: column conv as y1' = a0*x_up + x_center + a2*x_down
    (a_i = col_i/col_1 folded on host; col_1 absorbed into the matmul
    weights) — 2 DVE stt ops per sub-slab, no ACT center mul.
    factored=False: classic 3-op column pass (ACT center mul + 2 stt);
    used when some |col_1| is too small to divide by."""
    nc = bacc.Bacc(trn_type="TRN2", target_bir_lowering=False, debug=False)
    xin = nc.dram_tensor("xin", [B_LOC, C, H, W], F32, kind="ExternalInput").ap()
    wfold = nc.dram_tensor("wfold", [3, C, OUT], F32, kind="ExternalInput").ap()
    colk = nc.dram_tensor("colk", [C, 3], F32, kind="ExternalInput").ap()
    out = nc.dram_tensor("out", [B_LOC, OUT, H, W], F32, kind="ExternalOutput").ap()

    with tile.TileContext(nc) as tc, ExitStack() as ctx:
        wpool = ctx.enter_context(tc.tile_pool(name="weights", bufs=1))
        xpool = ctx.enter_context(tc.tile_pool(name="x", bufs=5))
        ypool = ctx.enter_context(tc.tile_pool(name="y1", bufs=6))
        opool = ctx.enter_context(tc.tile_pool(name="out", bufs=6))
        pspool = ctx.enter_context(tc.tile_pool(name="ps", bufs=8, space="PSUM"))

        # --- weights: DMA fp32, round to f32r on DVE (fp32r matmul operands
        # must be produced by a rounding compute op, not a DMA)
        w_f32 = wpool.tile([C, 3 * OUT], F32, tag="w32")
        for j in range(3):
            nc.sync.dma_start(w_f32[:, j * OUT:(j + 1) * OUT], wfold[j])
        w_r = wpool.tile([C, 3 * OUT], F32R, tag="wr")
        nc.vector.tensor_copy(w_r[:], w_f32[:])
        ck = wpool.tile([C, 3], F32, tag="ck")
        nc.sync.dma_start(ck[:], colk[:])

        def wj(j, oc):  # lhsT [C=128, O=128] for tap j, out-channel half oc
            return w_r[:, j * OUT + oc * 128: j * OUT + oc * 128 + 128]

        for rep in range(repeat):
            for b in range(B_LOC):
                for s in range(N_SLABS):
                    _slab(nc, tc, xin, out, xpool, ypool, opool, pspool,
                          wj, ck, b, s, rep, factored)
    nc.compile()
    return nc


def _slab(nc, tc, xin, out, xpool, ypool, opool, pspool, wj, ck, b, s, rep,
          factored):
                h0 = s * SLAB
                # --- x slab with halo: rows h0-1 .. h0+SLAB (SLAB+2 rows)
                x_t = xpool.tile([C, (SLAB + 2) * W], F32, tag="xs")
                if s == 0:
                    nc.gpsimd.memset(x_t[:, 0:W], 0.0)
                    nc.sync.dma_start(x_t[:, W:], xin[b, :, 0:SLAB + 1, :])
                elif s == N_SLABS - 1:
                    nc.gpsimd.memset(x_t[:, (SLAB + 1) * W:], 0.0)
                    nc.sync.dma_start(x_t[:, 0:(SLAB + 1) * W],
                                      xin[b, :, h0 - 1:H, :])
                else:
                    nc.sync.dma_start(x_t[:], xin[b, :, h0 - 1:h0 + SLAB + 1, :])

                # --- process in SUB-row sub-slabs so matmuls can start as
                # soon as a sub-slab's column pass is done (keeps PE dense
                # and warm instead of stalling on the whole-slab y1 chain).
                x3 = x_t[:].rearrange("c (h w) -> c h w", w=W)
                WP = W + 1
                RPC = CHUNK // W                  # rows per psum chunk (4)
                # out staging: half-slab tiles, DMA'd as soon as filled
                ot = [[opool.tile([C, SLAB // 2 * W], F32, tag="ot",
                                  name=f"ot_{rep}_{b}_{s}_{oc}_{hh}")
                       for hh in range(2)] for oc in range(2)]  # [oc][half]

                for ss in range(N_SUB):
                    base = ss * SUB
                    # y1 sub-tile: SUB data rows, row stride W+1; y1[h][w]
                    # lives at offset 1 + h*WP + w, and the inter-row pad
                    # column (offset h*WP) is zeroed => the row conv gets
                    # zero-pad edge semantics with full-width, fp32r-legal
                    # access patterns: tap j of rows r0..r0+3 is
                    # ypj[j][:, r0:r0+4, 0:W].
                    y1 = ypool.tile([C, (SUB + 1) * WP + 2], F32R, tag="y1")
                    yp = y1[:, 0:(SUB + 1) * WP].rearrange(
                        "c (h w) -> c h w", w=WP)
                    ypj = [y1[:, j:j + (SUB + 1) * WP]
                           .rearrange("c (h w) -> c h w", w=WP)
                           for j in range(3)]
                    nc.vector.memset(yp[:, :, 0:1].bitcast(F32), 0.0)
                    yd = yp[:, 0:SUB, 1:WP]       # data view [C, SUB, W]
                    if factored:
                        # y1' = (x_up * a0) + x_center ; y1' += x_down * a2
                        nc.vector.scalar_tensor_tensor(
                            yd, x3[:, base:base + SUB, :], ck[:, 0:1],
                            x3[:, base + 1:base + SUB + 1, :],
                            op0=mybir.AluOpType.mult, op1=mybir.AluOpType.add)
                        nc.vector.scalar_tensor_tensor(
                            yd, x3[:, base + 2:base + SUB + 2, :], ck[:, 2:3],
                            yd,
                            op0=mybir.AluOpType.mult, op1=mybir.AluOpType.add)
                    else:
                        nc.scalar.activation(
                            yd, x3[:, base + 1:base + SUB + 1, :],
                            mybir.ActivationFunctionType.Copy, scale=ck[:, 1:2])
                        nc.vector.scalar_tensor_tensor(
                            yd, x3[:, base:base + SUB, :], ck[:, 0:1], yd,
                            op0=mybir.AluOpType.mult, op1=mybir.AluOpType.add)
                        nc.vector.scalar_tensor_tensor(
                            yd, x3[:, base + 2:base + SUB + 2, :], ck[:, 2:3],
                            yd,
                            op0=mybir.AluOpType.mult, op1=mybir.AluOpType.add)

                    # row conv + pointwise folded into PE (f32r matmuls)
                    for oc in range(2):
                        for qq in range(SUB // RPC):
                            ps = pspool.tile([128, CHUNK], F32, tag="ps")
                            r0 = qq * RPC
                            for jx, j in enumerate((0, 1, 2)):
                                nc.tensor.matmul(
                                    ps[:], wj(j, oc),
                                    ypj[j][:, r0:r0 + RPC, 0:W],
                                    start=(jx == 0), stop=(jx == 2))
                            qg = ss * (SUB // RPC) + qq   # global chunk 0..7
                            half = qg // (N_CHUNK // 2)
                            qh = qg % (N_CHUNK // 2)
                            dst = ot[oc][half][:, qh * CHUNK:(qh + 1) * CHUNK]
                            if factored:
                                dve_evac = ss == 3 and oc == 1 and qq == 1
                            else:
                                dve_evac = (oc * (SUB // RPC) + qq) % 4 == 3
                            if dve_evac:
                                nc.vector.tensor_copy(dst, ps[:])
                            else:
                                nc.scalar.copy(dst, ps[:])
                    if ss % (N_SUB // 2) == N_SUB // 2 - 1:
                        half = ss // (N_SUB // 2)
                        hr = h0 + half * (SLAB // 2)
                        for oc in range(2):
                            # never issue output DMAs from SP (they would
                            # block the input-prefetch issue stream). ACT
                            # (HWDGE, faster) issues tiles it evacuated
                            # itself; the DVE-evacuated tile goes via GpSimd
                            # so ACT doesn't stall on a DVE wait.
                            if factored and oc == 1 and half == 1:
                                eng = nc.gpsimd
                            elif factored:
                                eng = nc.scalar
                            else:
                                eng = nc.gpsimd
                            eng.dma_start(
                                out[b, oc * 128:(oc + 1) * 128,
                                    hr:hr + SLAB // 2, :], ot[oc][half][:])


def host_prep(col_kernel, row_kernel, pw_weight):
    """Fold weights on the host. Returns (factored, wfold [3,C,OUT],
    colk [C,3])."""
    colk3 = np.asarray(col_kernel, dtype=np.float64).reshape(C, 3)
    rowk3 = np.asarray(row_kernel, dtype=np.float64).reshape(C, 3)
    pw = np.asarray(pw_weight, dtype=np.float64)

    c1 = colk3[:, 1]
    factored = bool(np.abs(c1).min() > 1e-3)
    # Wj[c, o] = pw[o,c] * row[c,j]  (times c1[c] when factored)
    wfold = pw.T[None, :, :] * rowk3.T[:, :, None]      # [3, C, OUT]
    if factored:
        wfold = wfold * c1[None, :, None]
        ck = np.stack([colk3[:, 0] / c1, c1, colk3[:, 2] / c1], axis=1)
    else:
        ck = colk3
    return (factored,
            np.ascontiguousarray(wfold).astype(np.float32),
            np.ascontiguousarray(ck).astype(np.float32))


def kernel(x, col_kernel, row_kernel, pw_weight, trace=False):
    global LAST_EXEC_NS, _CACHED_NC
    x = np.ascontiguousarray(np.asarray(x, dtype=np.float32))
    factored, wfold, colk3 = host_prep(col_kernel, row_kernel, pw_weight)

    if _CACHED_NC is None or _CACHED_NC[1] != factored:
        _CACHED_NC = (_build(factored=factored), factored)
    nc = _CACHED_NC[0]

    in_maps = [
        {"xin": np.ascontiguousarray(x[i * B_LOC:(i + 1) * B_LOC]),
         "wfold": wfold, "colk": colk3}
        for i in range(N_CORES)
    ]
    res = run_bass_kernel_spmd(nc, in_maps, list(range(N_CORES)), trace=trace)
    LAST_EXEC_NS = res.exec_time_ns
    return np.concatenate([res.results[i]["out"] for i in range(N_CORES)],
                          axis=0)
